# revision 1
# baseline (speedup 1.0000x reference)
"""Trainium2 Bass kernel for CustomMultiHeadAttention (single-query pooled attention).

Reference computation (B=32, S=1024, D=256, H=8):
    keys   = (x @ Wk + bk).reshape(B,S,H,D)
    values = (x @ Wv + bv).reshape(B,S,H,D)
    scores = einsum('bshd,hd->bsh', keys, query)
    attn   = softmax(scores, axis=1)           # over S
    pooled = einsum('bsh,bshd->bhd', attn, values).reshape(B, H*D)
    out    = pooled @ Wo + bo

Algebraic restructure (exact in real arithmetic):
    q_proj[e,h] = sum_d Wk[e, h*D+d] * query[h,d]        # [256, 8]
    scores[b,s,h] = x[b,s,:] @ q_proj[:,h]  (+ const(h) from bk -> cancels in softmax)
    attnu = exp(scores - 64)                             # const shift; softmax invariant
    ctx[b,h,e]  = sum_s attnu[b,s,h] * x[b,s,e];  Z[b,h] = sum_s attnu[b,s,h]
    pooled[b,h,:] = (ctx[b,h,:]/Z[b,h]) @ Wv_h + bv_h    # sum_s attn = 1
    out = pooled @ Wo + (bv @ Wo + bo)

This removes both [B*S,256]x[256,2048] projections; the kernel is memory-bound.
Z is obtained free as an extra all-ones column appended to x in the ctx matmul.
Scores use exact fp32 matmuls (cheap: N=8); the post-softmax path uses float32r.
Sharding: data-parallel over batch, 4 batches per core on 8 cores.

Layout note: PE matmul operands/outputs need base partition in {0,32,64}, so
local batches 0..2 sit at partition offsets 0/32/64 and batch 3 uses a second
free-dim slab at offset 0 (only relevant for the tiny [8 x *] ctx tiles).
"""

import sys

sys.path.insert(0, "/opt/trn_rl_repo")

import numpy as np

import concourse.bass as bass
import concourse.mybir as mybir
import concourse.tile as tile
from concourse import bacc
from concourse.bass_utils import run_bass_kernel_spmd
from concourse.masks import make_identity

F32 = mybir.dt.float32
F32R = mybir.dt.float32r

B, S, D, H = 32, 1024, 256, 8
NCORES = 8
BL = B // NCORES      # local batches per core = 4
ST = S // 128         # s-tiles per batch = 8
KD = 2                # 256 = 2 k-tiles of 128 over the D (input dim) axis
KHD = (H * D) // 128  # 16 k-tiles over the H*D axis
SHIFT = 64.0          # constant score shift before exp (softmax-invariant)

def build_program():
    nc = bacc.Bacc("TRN2", target_bir_lowering=False, debug=False)

    xn_d = nc.dram_tensor("xn", [BL, S, D + 2], F32R, kind="ExternalInput")
    wk_d = nc.dram_tensor("wk", [D, H * D], F32, kind="ExternalInput")
    wv_d = nc.dram_tensor("wv", [D, H * D], F32R, kind="ExternalInput")
    wo_d = nc.dram_tensor("wo", [H * D, D], F32R, kind="ExternalInput")
    q_d = nc.dram_tensor("q", [H, D], F32, kind="ExternalInput")
    bv_d = nc.dram_tensor("bv", [H * D], F32, kind="ExternalInput")
    bo_d = nc.dram_tensor("bo", [D], F32R, kind="ExternalInput")
    on_d = nc.dram_tensor("on", [1, BL], F32R, kind="ExternalInput")
    out_d = nc.dram_tensor("out", [BL, D], F32, kind="ExternalOutput")

    with tile.TileContext(nc) as tc:
        with (
            tc.tile_pool(name="big", bufs=1) as big,
            tc.tile_pool(name="sm", bufs=1) as sm,
            tc.tile_pool(name="ps", bufs=1, space=bass.MemorySpace.PSUM) as ps,
            tc.tile_pool(name="pst", bufs=2, space=bass.MemorySpace.PSUM) as pst,
        ):
            # ---- SBUF allocations -------------------------------------
            xn_sb = big.tile([128, BL, ST, D + 2], F32R)  # x natural + 2 ones cols
            xt_sb = big.tile([128, KD, BL, S], F32)       # x transposed: p=e%128
            wk_sb = big.tile([128, KD, H * D], F32)
            wv_sb = big.tile([128, KD, H * D], F32R)
            wo_sb = big.tile([128, KHD, D], F32R)
            qrep = big.tile([128, H * D], F32)            # query replicated
            qsmall = sm.tile([1, H * D], F32)
            tmp = big.tile([128, KD, H * D], F32)         # wk * qrep scratch

            qp = sm.tile([128, KD, H], F32)               # q_proj [e, h]
            attn_sb = sm.tile([128, BL, ST, H], F32R)     # exp(scores-SHIFT) [s, h]
            recip = sm.tile([H, BL, 1], F32)              # 1/Z per (h, b)
            ctx_sb = sm.tile([H, BL, D], F32)             # [h, b, e]
            ctxT_sb = sm.tile([128, KD, BL, H], F32R)     # [e%128, eh, b, h]
            pooledT_sb = sm.tile([128, KHD, BL], F32R)    # [(hd)%128, ktile, b]
            bvn_sb = sm.tile([KHD, 128], F32)             # bv natural [k, p]
            bvT_sb = sm.tile([128, KHD], F32R)
            bo_sb = sm.tile([1, D], F32R)
            bias_sb = sm.tile([1, D], F32R)               # bv @ Wo + bo
            ones_sb = sm.tile([1, BL], F32R)
            ident = sm.tile([16, 16], F32)
            ident128 = sm.tile([128, 128], F32)
            negs = sm.tile([128, 1], F32)                 # -SHIFT bias for exp
            out_sb = sm.tile([BL, D], F32)

            # ---- DMA loads -------------------------------------------
            nc.sync.dma_start(
                qsmall[:], q_d[:].rearrange("h d -> () (h d)")
            )
            nc.gpsimd.partition_broadcast(qrep[:], qsmall[:])
            nc.sync.dma_start(
                wk_sb[:], wk_d[:].rearrange("(k p) f -> p k f", p=128)
            )
            for b in range(BL):
                nc.sync.dma_start(
                    xn_sb[:, b, :, :],
                    xn_d[b].rearrange("(t p) e -> p t e", p=128),
                )
            nc.sync.dma_start(
                wv_sb[:], wv_d[:].rearrange("(k p) f -> p k f", p=128)
            )
            for kh in range(2):
                nc.sync.dma_start(
                    wo_sb[:, kh * 8:(kh + 1) * 8, :],
                    wo_d[kh * 1024:(kh + 1) * 1024, :]
                    .rearrange("(k p) n -> p k n", p=128),
                )
            nc.sync.dma_start(bvn_sb[:], bv_d[:].rearrange("(k p) -> k p", p=128))
            nc.sync.dma_start(bo_sb[:], bo_d[:].rearrange("d -> () d"))
            nc.sync.dma_start(ones_sb[:], on_d[:])

            make_identity(nc, ident[:])
            make_identity(nc, ident128[:])
            nc.vector.memset(negs[:], -SHIFT)

            # ---- q_proj[e,h] = sum_d Wk[e, h*D+d] * query[h,d] --------
            nc.vector.tensor_mul(
                tmp[:],
                wk_sb[:],
                qrep[:].rearrange("p f -> p () f").broadcast_to([128, KD, H * D]),
            )
            nc.vector.reduce_sum(
                qp[:],
                tmp[:].rearrange("p k (h d) -> p k h d", d=D),
                axis=mybir.AxisListType.X,
            )

            # ---- transpose x on chip: xt[e, s] per (b, eh) (PE, fp32) -
            # 4 transposes share one PSUM bank -> one batched DVE copy
            for b in range(BL):
                for tp2 in range(ST // 2):       # pairs of s-tiles
                    xtp = pst.tile([128, 2, 2, 128], F32, tag="xtp")
                    for toff in range(2):
                        t = tp2 * 2 + toff
                        for eh in range(KD):
                            nc.tensor.transpose(
                                xtp[:, toff, eh, :],
                                xn_sb[:, b, t, eh * 128:(eh + 1) * 128].bitcast(F32),
                                ident128[:],
                            )
                    # dest [p, eh, s(2x128)]; src permuted [p, eh, toff, 128]
                    nc.vector.tensor_copy(
                        xt_sb[:, :, b, tp2 * 256:(tp2 + 1) * 256]
                        .rearrange("p k (o s) -> p k o s", o=2),
                        xtp[:].rearrange("p o k s -> p k o s"),
                    )

            # ---- scores[s, h] per (b, s-tile) = xt_tile.T @ q_proj ----
            # out[s, h] = sum_e xt[e, s] * qp[e, h]; exact fp32 (N=8 so cheap)
            scores_ps = ps.tile([128, BL, ST, H], F32, tag="scores")
            for b in range(BL):
                for t in range(ST):
                    for k in range(KD):
                        nc.tensor.matmul(
                            scores_ps[:, b, t, :],
                            xt_sb[:, k, b, t * 128:(t + 1) * 128],
                            qp[:, k, :],
                            start=(k == 0),
                            stop=(k == KD - 1),
                        )
                # exp(scores - SHIFT) -> unnormalized attention weights
                nc.scalar.activation(
                    attn_sb[:, b, :, :],
                    scores_ps[:, b, :, :],
                    mybir.ActivationFunctionType.Exp,
                    bias=negs[:],
                )

            # ---- ctx[h, e] & Z per batch: attnu.T @ [x | 1] (PE) ------
            for b in range(BL):
                ctx_ps = pst.tile([H, 512], F32, tag="ctx")
                for t in range(ST):
                    nc.tensor.matmul(
                        ctx_ps[:, 0:D + 2],
                        attn_sb[:, b, t, :],
                        xn_sb[:, b, t, :],
                        start=(t == 0),
                        stop=(t == ST - 1),
                    )
                # 1/Z from the ones column, then fold into ctx
                nc.vector.reciprocal(recip[:, b, :], ctx_ps[:, D:D + 1])
                nc.vector.tensor_scalar_mul(
                    ctx_sb[:, b, :],
                    ctx_ps[:, 0:D],
                    recip[:, b, :],
                )

            # ---- ctxT[e, (b,h)] via PE transpose ----------------------
            for b in range(BL):
                for eh in range(KD):
                    ctp = pst.tile([128, H], F32, tag="tp")
                    nc.tensor.transpose(
                        ctp[:],
                        ctx_sb[:, b, eh * 128:(eh + 1) * 128],
                        ident[:H, :H],
                    )
                    nc.vector.tensor_copy(ctxT_sb[:, eh, b, :], ctp[:])

            # ---- pooledT[(h d), b] = Wv_h.T @ ctx_h.T (PE, f32r) ------
            pooledT_ps = pst.tile([128, KHD, BL], F32, tag="tp")
            for h in range(H):
                for dh in range(2):
                    for k in range(KD):
                        nc.tensor.matmul(
                            pooledT_ps[:, h * 2 + dh, :],
                            wv_sb[:, k, h * D + dh * 128: h * D + (dh + 1) * 128],
                            ctxT_sb[:, k, :, h],
                            start=(k == 0),
                            stop=(k == KD - 1),
                        )
            nc.vector.tensor_copy(pooledT_sb[:], pooledT_ps[:])

            # ---- bias_total = bv @ Wo + bo (PE) -----------------------
            bvt_ps = pst.tile([128, KHD], F32, tag="tp")
            nc.tensor.transpose(bvt_ps[:], bvn_sb[:], ident[:KHD, :KHD])
            nc.vector.tensor_copy(bvT_sb[:], bvt_ps[:])

            bias_ps = ps.tile([1, D], F32, tag="fin")
            for k in range(KHD):
                nc.tensor.matmul(
                    bias_ps[:],
                    bvT_sb[:, k:k + 1],
                    wo_sb[:, k, :],
                    start=(k == 0),
                    stop=False,
                )
            nc.tensor.matmul(
                bias_ps[:],
                ones_sb[0:1, 0:1],
                bo_sb[:],
                start=False,
                stop=True,
            )
            nc.vector.tensor_copy(bias_sb[:], bias_ps[:])

            # ---- out[b, :] = pooled_flat @ Wo + bias_total (PE, f32r) -
            out_ps = ps.tile([BL, D], F32, tag="scores")
            for k in range(KHD):
                nc.tensor.matmul(
                    out_ps[:],
                    pooledT_sb[:, k, :],
                    wo_sb[:, k, :],
                    start=(k == 0),
                    stop=False,
                )
            nc.tensor.matmul(
                out_ps[:],
                ones_sb[:],
                bias_sb[:],
                start=False,
                stop=True,
            )
            nc.vector.tensor_copy(out_sb[:], out_ps[:])
            nc.sync.dma_start(out_d[:], out_sb[:])

    nc.compile()
    return nc


_NC_CACHE = []


def get_nc():
    if not _NC_CACHE:
        _NC_CACHE.append(build_program())
    return _NC_CACHE[0]


def make_in_maps(x, Wk, bk, Wv, bv, query, Wo, bo):
    x = np.ascontiguousarray(x, dtype=np.float32)
    xn1 = np.concatenate(
        [x, np.ones((x.shape[0], x.shape[1], 2), np.float32)], axis=2
    )
    wk = np.ascontiguousarray(Wk, dtype=np.float32)
    wv = np.ascontiguousarray(Wv, dtype=np.float32)
    wo = np.ascontiguousarray(Wo, dtype=np.float32)
    q = np.ascontiguousarray(query, dtype=np.float32)
    bvv = np.ascontiguousarray(bv, dtype=np.float32)
    bob = np.ascontiguousarray(bo, dtype=np.float32)
    in_maps = []
    for c in range(NCORES):
        sl = slice(c * BL, (c + 1) * BL)
        in_maps.append(
            {
                "xn": xn1[sl],
                "wk": wk,
                "wv": wv,
                "wo": wo,
                "q": q,
                "bv": bvv,
                "bo": bob,
                "on": np.ones((1, BL), np.float32),
            }
        )
    return in_maps


def kernel(x, Wk, bk, Wv, bv, query, Wo, bo):
    nc = get_nc()
    in_maps = make_in_maps(x, Wk, bk, Wv, bv, query, Wo, bo)
    res = run_bass_kernel_spmd(nc, in_maps, core_ids=list(range(NCORES)))
    return np.concatenate([res.results[c]["out"] for c in range(NCORES)], axis=0)



# revision 2
# speedup vs baseline: 1.0431x; 1.0431x over previous
"""Trainium2 Bass kernel for CustomMultiHeadAttention (single-query pooled attention).

Reference computation (B=32, S=1024, D=256, H=8):
    keys   = (x @ Wk + bk).reshape(B,S,H,D)
    values = (x @ Wv + bv).reshape(B,S,H,D)
    scores = einsum('bshd,hd->bsh', keys, query)
    attn   = softmax(scores, axis=1)           # over S
    pooled = einsum('bsh,bshd->bhd', attn, values).reshape(B, H*D)
    out    = pooled @ Wo + bo

Algebraic restructure (exact in real arithmetic):
    qp[e,h]   = sum_d Wk[e, h*D+d] * query[h,d]        # [256, 8]   (host fold)
    scores[b,s,h] = x[b,s,:] @ qp[:,h]  (+ const(h) from bk -> cancels in softmax)
    attnu     = exp(scores - 64)                        # const shift; softmax invariant
    ctx[b,h,e] = sum_s attnu[b,s,h] * x[b,s,e];  Z[b,h] = sum_s attnu[b,s,h]
    U[h]      = Wv_h @ Wo_h                             # [8,256,256] (host fold)
    out[b]    = sum_h (ctx[b,h,:]/Z[b,h]) @ U[h] + (bv @ Wo + bo)

This removes both [B*S,256]x[256,2048] projections; the kernel is memory-bound.
Z is obtained free as an extra all-ones column appended to x in the ctx matmul.

Single-core layout: per-dispatch overhead on this (axon-tunneled) fabric scales
with the number of cores in a dispatch (~65us per extra core) and dwarfs the
device time of per-core shards, so one core running the whole problem
minimizes steady-state per-execution time.

Precision assignment (numpy-simulated rel err 2.2e-3 vs fp32 reference,
reproduced exactly on hardware):
  - scores path: x ships as fp16 e-major halves and is transposed by the
    DMA xbar (scores need e on partitions); qp fp16; fp32 PSUM accumulate.
  - attn: bf16 (needs fp32 exponent range for exp(scores-64); 8-bit mantissa
    only perturbs softmax weights ~0.4%).
  - value path: natural x partition-major bf16 (matches attn's dtype), ctx
    accumulated in fp32, normalized z cast to fp16, U fp16.
Transposing and copy DMAs must share one HWDGE ring (concurrent xbar-transpose
and copy DMA on separate rings corrupts data - measured), so the two whole-x
transposes issue once up front and the natural loads stream behind them.
"""

import sys

sys.path.insert(0, "/opt/trn_rl_repo")

import numpy as np
import ml_dtypes

import concourse.bass as bass
import concourse.mybir as mybir
import concourse.tile as tile
from concourse import bacc
from concourse.bass_utils import run_bass_kernel_spmd
from concourse.masks import make_identity

F32 = mybir.dt.float32
F16 = mybir.dt.float16
BF16 = mybir.dt.bfloat16

B, S, D, H = 32, 1024, 256, 8
ST = S // 128          # s-tiles per batch = 8
E2 = D + 4             # natural-layout cols: 256 data + ones col + pad
CB = 4                 # batches per chunk
NCH = B // CB          # chunks = 8
SHIFT = 64.0           # constant score shift before exp (softmax-invariant)
KD = 2                 # 256 = 2 k-tiles of 128 over the e axis
NCORES = 1
BL = B


def build_program():
    nc = bacc.Bacc("TRN2", target_bir_lowering=False, debug=False)

    xta_d = nc.dram_tensor("xta", [B * S, 128], F16, kind="ExternalInput")
    xtb_d = nc.dram_tensor("xtb", [B * S, 128], F16, kind="ExternalInput")
    xn_d = nc.dram_tensor("xn", [128, B, ST, E2], BF16, kind="ExternalInput")
    qp_d = nc.dram_tensor("qp", [128, KD, H], F16, kind="ExternalInput")
    u_d = nc.dram_tensor("u", [128, KD * H, D], F16, kind="ExternalInput")
    bias_d = nc.dram_tensor("bias", [B, D], F32, kind="ExternalInput")
    out_d = nc.dram_tensor("out", [B, D], F32, kind="ExternalOutput")

    with tile.TileContext(nc) as tc:
        with (
            tc.tile_pool(name="wt", bufs=1) as wt,
            tc.tile_pool(name="xnp", bufs=3) as xnp,
            tc.tile_pool(name="smp", bufs=2) as smp,
            tc.tile_pool(name="psc", bufs=3, space=bass.MemorySpace.PSUM) as psc,
            tc.tile_pool(name="ps2", bufs=2, space=bass.MemorySpace.PSUM) as ps2,
            tc.tile_pool(name="ps1", bufs=1, space=bass.MemorySpace.PSUM) as ps1,
        ):
            # ---- persistent SBUF ----
            qp_sb = wt.tile([128, KD, H], F16)
            u_sb = wt.tile([128, KD * H, D], F16)
            bias_sb = wt.tile([B, D], F32)
            ctxT_sb = wt.tile([128, KD, H, B], F16)   # [e%128, eh, h, b]
            negs = wt.tile([128, 1], F32)
            ident = wt.tile([16, 16], F32)
            out_sb = wt.tile([B, D], F32)
            xt_w = wt.tile([128, KD, B * S], F16)     # x transposed, whole

            nc.sync.dma_start(qp_sb[:], qp_d[:])
            nc.sync.dma_start(u_sb[:], u_d[:])
            nc.sync.dma_start(bias_sb[:], bias_d[:])
            make_identity(nc, ident[:])
            nc.vector.memset(negs[:], -SHIFT)

            # Whole-x xbar transposes first (one xbar-mode transition), then
            # the natural loads stream behind them on the same ring.
            nc.sync.dma_start_transpose(xt_w[:, 0, :], xta_d[:])
            nc.sync.dma_start_transpose(xt_w[:, 1, :], xtb_d[:])

            for ch in range(NCH):
                xn_t = xnp.tile([128, CB, ST, E2], BF16, tag="xn")
                nc.sync.dma_start(xn_t[:], xn_d[:, ch * CB:(ch + 1) * CB])

                attn_ts = []
                for lb in range(CB):
                    # ---- scores[s, h]; k-accumulated over the e halves ----
                    sc_ps = psc.tile([128, ST, H], F32, tag="sc")
                    for t in range(ST):
                        col = ((ch * CB + lb) * ST + t) * 128
                        for k in range(KD):
                            nc.tensor.matmul(
                                sc_ps[:, t, :],
                                xt_w[:, k, col:col + 128],
                                qp_sb[:, k, :],
                                start=(k == 0),
                                stop=(k == KD - 1),
                            )
                    # exp(scores - SHIFT) -> unnormalized attention (bf16)
                    attn_lb = smp.tile([128, ST, H], BF16, tag="attn")
                    nc.scalar.activation(
                        attn_lb[:],
                        sc_ps[:],
                        mybir.ActivationFunctionType.Exp,
                        bias=negs[:],
                    )
                    attn_ts.append(attn_lb)

                # ---- ctx[h, :] & Z per batch: attnu.T @ [x | 1] ----
                for lb in range(CB):
                    b = ch * CB + lb
                    ctx_ps = ps2.tile([H, E2], F32, tag="ctx")
                    for t in range(ST):
                        nc.tensor.matmul(
                            ctx_ps[:],
                            attn_ts[lb][:, t, :],
                            xn_t[:, lb, t, :],
                            start=(t == 0),
                            stop=(t == ST - 1),
                        )
                    recip = smp.tile([H, 1], F32, tag="recip")
                    nc.vector.reciprocal(recip[:], ctx_ps[:, D:D + 1])
                    ctx_sb = smp.tile([H, D], F32, tag="ctxsb")
                    nc.vector.tensor_scalar_mul(ctx_sb[:], ctx_ps[:, 0:D], recip[:])
                    for eh in range(KD):
                        ctp = ps2.tile([128, H], F32, tag="tp")
                        nc.tensor.transpose(
                            ctp[:],
                            ctx_sb[:, eh * 128:(eh + 1) * 128],
                            ident[:H, :H],
                        )
                        nc.vector.tensor_copy(ctxT_sb[:, eh, :, b], ctp[:])

            # ---- out[b, :] = sum_kk zT_kk.T @ U_kk + bias ----
            out_ps = ps1.tile([B, D], F32, tag="out")
            for kk in range(KD * H):
                h, eh = kk // KD, kk % KD
                nc.tensor.matmul(
                    out_ps[:],
                    ctxT_sb[:, eh, h, :],
                    u_sb[:, kk, :],
                    start=(kk == 0),
                    stop=(kk == KD * H - 1),
                )
            nc.vector.tensor_add(out_sb[:], out_ps[:], bias_sb[:])
            nc.sync.dma_start(out_d[:], out_sb[:])

    nc.compile()
    return nc


_NC_CACHE = []


def get_nc():
    if not _NC_CACHE:
        _NC_CACHE.append(build_program())
    return _NC_CACHE[0]


def make_in_maps(x, Wk, bk, Wv, bv, query, Wo, bo):
    x = np.ascontiguousarray(x, dtype=np.float32)
    Wk = np.asarray(Wk, np.float32)
    Wv = np.asarray(Wv, np.float32)
    Wo = np.asarray(Wo, np.float32)
    query = np.asarray(query, np.float32)
    bv = np.asarray(bv, np.float32)
    bo = np.asarray(bo, np.float32)

    # host weight folds (0.4% of the reference FLOPs; bk cancels in softmax)
    qp = np.stack([Wk[:, h * D:(h + 1) * D] @ query[h] for h in range(H)], axis=1)
    U = np.stack([Wv[:, h * D:(h + 1) * D] @ Wo[h * D:(h + 1) * D] for h in range(H)])
    bias = bv @ Wo + bo

    xta = np.ascontiguousarray(x[:, :, :128].reshape(B * S, 128)).astype(np.float16)
    xtb = np.ascontiguousarray(x[:, :, 128:].reshape(B * S, 128)).astype(np.float16)

    xr = x.reshape(B, ST, 128, D)
    xn = np.zeros((128, B, ST, E2), dtype=ml_dtypes.bfloat16)
    xn[:, :, :, :D] = xr.transpose(2, 0, 1, 3).astype(ml_dtypes.bfloat16)
    xn[:, :, :, D] = 1.0

    qph = np.ascontiguousarray(
        qp.reshape(KD, 128, H).transpose(1, 0, 2)).astype(np.float16)
    # u[p, kk, n] with kk = h*KD + eh, e = eh*128 + p
    uh = np.ascontiguousarray(
        U.reshape(H, KD, 128, D).transpose(2, 0, 1, 3).reshape(128, KD * H, D)
    ).astype(np.float16)
    biash = np.ascontiguousarray(np.broadcast_to(bias, (B, D))).astype(np.float32)

    return [{"xta": xta, "xtb": xtb, "xn": xn, "qp": qph, "u": uh, "bias": biash}]


def kernel(x, Wk, bk, Wv, bv, query, Wo, bo):
    nc = get_nc()
    in_maps = make_in_maps(x, Wk, bk, Wv, bv, query, Wo, bo)
    res = run_bass_kernel_spmd(nc, in_maps, core_ids=[0])
    return np.asarray(res.results[0]["out"], dtype=np.float32)
